# revision 10
# baseline (speedup 1.0000x reference)
# Trainium2 Bass kernel for the ContextBlock problem.
#
# Reference computation (per sample b):
#   xc    = concat(x0..x3)            [C=1024, HW=4096]
#   attn  = softmax(wm @ xc)          [HW]
#   ctx   = xc @ attn                 [C]
#   mul   = residual-gated MLP stack (sigmoid branch)   [C]
#   add   = residual-gated MLP stack (linear branch)    [C]
#   out   = sum_l (x_l * mul_l + add_l)                 [CL=256, HW]
#
# Distribution: data-parallel over batch, one sample per NeuronCore (B=8).
# No collectives required.
#
# Per-core dataflow (chunk-pipelined):
#   phase1: per 512-col chunk: logits via replicated-wm matmuls (the PSUM
#           result is partition-broadcast for free), exp on Scalar with
#           per-chunk accum (softmax denominator), then Vector
#           tensor_tensor_reduce accumulates the context dot-products.
#           Overlaps PE / Scalar / Vector / x DMA.
#   gates:  W1 in fp8 e4m3 (x64 host pre-scale; LN is scale-invariant so
#           only b1 and EPS are rescaled), W2 bf16. All weight DMA issued
#           up-front behind x. Layer pairs interleaved for PE pipelining.
#   pass3:  diag(mul) @ x accumulated over levels in PSUM, bias=add-sum,
#           bf16 output staging + DMA (host upcasts to f32).

import numpy as np
import ml_dtypes
from contextlib import ExitStack

import concourse.bass as bass
import concourse.bacc as bacc
import concourse.mybir as mybir
import concourse.tile as tile

BF = mybir.dt.bfloat16
F32 = mybir.dt.float32
F8 = mybir.dt.float8e4
AF = mybir.ActivationFunctionType
ALU = mybir.AluOpType
AX = mybir.AxisListType

B, L, CL, H, W = 8, 4, 256, 64, 64
C = L * CL          # 1024
HW = H * W          # 4096
P = C // 4          # 256
R = 2
EPS = 1e-5
W1SC = 64.0         # fp8 pre-scale for W1 (and b1); LN makes it a no-op
EPS_S = EPS * W1SC * W1SC
NJ = C // 128       # 8   c-slabs
NCH = HW // 512     # 8   512-col chunks
NCORES = 8

_CACHE = {}


def _build_nc():
    nc = bacc.Bacc()

    x_d = nc.dram_tensor("x", [C, HW], BF, kind="ExternalInput")
    wmr_d = nc.dram_tensor("wmr", [128, NJ, 128], BF, kind="ExternalInput")
    rhsi_d = nc.dram_tensor("rhsi", [128, 128], BF, kind="ExternalInput")
    ones_d = nc.dram_tensor("ones", [128, 128], F32, kind="ExternalInput")
    wg1_d = nc.dram_tensor("wg1", [NJ, 128, 4, 1024], F8, kind="ExternalInput")
    wg2_d = nc.dram_tensor("wg2", [128, 4, 2048], BF, kind="ExternalInput")
    sm_d = nc.dram_tensor("smalls", [128, 128], F32, kind="ExternalInput")
    out_d = nc.dram_tensor("out", [CL, HW], BF, kind="ExternalOutput")

    with tile.TileContext(nc) as tc, ExitStack() as ctx:
        resid = ctx.enter_context(tc.tile_pool(name="resid", bufs=1))
        spool = ctx.enter_context(tc.tile_pool(name="spool", bufs=1))
        scr = ctx.enter_context(tc.tile_pool(name="scr", bufs=2))
        stpool = ctx.enter_context(tc.tile_pool(name="stage", bufs=4))
        dpool = ctx.enter_context(tc.tile_pool(name="diag", bufs=1))
        psh = ctx.enter_context(
            tc.tile_pool(name="psh", bufs=2, space=bass.MemorySpace.PSUM)
        )
        psg = ctx.enter_context(
            tc.tile_pool(name="psg", bufs=2, space=bass.MemorySpace.PSUM)
        )

        # ---- resident tiles ----
        x_sb = resid.tile([128, NJ, HW], BF, tag="x")
        wmr = resid.tile([128, NJ, 128], BF, tag="wmr")
        rhsi = resid.tile([128, 128], BF, tag="rhsi")
        ones = resid.tile([128, 128], F32, tag="ones")
        wg1 = resid.tile([128, NJ, 4, 1024], F8, tag="wg1")
        wg2 = resid.tile([128, 4, 2048], BF, tag="wg2")
        sm = resid.tile([128, 128], F32, tag="sm")
        e_bc = resid.tile([128, NCH, 512], BF, tag="e_bc")

        # x first (half-major so phase1 can start early); weights on gpsimd
        nc.scalar.dma_start(wmr[:], wmr_d[:])
        nc.scalar.dma_start(ones[:], ones_d[:])
        nc.scalar.dma_start(sm[:], sm_d[:])
        nc.scalar.dma_start(rhsi[:], rhsi_d[:])
        for ci in range(2):
            for j in range(NJ):
                nc.sync.dma_start(
                    x_sb[:, j, 2048 * ci:2048 * (ci + 1)],
                    x_d[128 * j:128 * (j + 1), 2048 * ci:2048 * (ci + 1)],
                )
        for j in range(NJ):
            nc.gpsimd.dma_start(wg1[:, j, :, :], wg1_d[j])
        nc.gpsimd.dma_start(wg2[:], wg2_d[:])

        ones_col = ones[:, 0:1]
        ones_row = ones[0:1, 0:128]

        # ---- phase 1: logits (partition-broadcast) + exp + context ----
        rowsums = spool.tile([128, NCH], F32, tag="rowsums")
        v0parts = spool.tile([128, NCH * NJ], F32, tag="v0parts")
        with tc.tile_pool(name="ps1", bufs=2,
                          space=bass.MemorySpace.PSUM) as ps1:
            for cj in range(NCH):
                lgbc = ps1.tile([128, 512], F32, tag="lgbc")
                for j in range(NJ):
                    nc.tensor.matmul(
                        lgbc[:],
                        wmr[:, j, :],
                        x_sb[:, j, 512 * cj:512 * (cj + 1)],
                        start=(j == 0), stop=(j == NJ - 1),
                    )
                # e = exp(logits) broadcast over partitions; accum -> Z chunk
                nc.scalar.activation(
                    e_bc[:, cj, :], lgbc[:], AF.Exp,
                    accum_out=rowsums[:, cj:cj + 1],
                )
                for j in range(NJ):
                    eng = nc.vector
                    t = scr.tile([128, 512], BF, tag="ttrv")
                    eng.scalar_tensor_tensor(
                        out=t[:],
                        in0=x_sb[:, j, 512 * cj:512 * (cj + 1)],
                        scalar=1.0,
                        in1=e_bc[:, cj, :],
                        op0=ALU.mult,
                        op1=ALU.mult,
                        accum_out=v0parts[:, NJ * cj + j:NJ * cj + j + 1],
                    )

        zcol = spool.tile([128, 2], F32, tag="zcol")
        nc.vector.reduce_sum(out=zcol[:, 0:1], in_=rowsums[:], axis=AX.X)
        nc.vector.reciprocal(zcol[:, 1:2], zcol[:, 0:1])
        v0 = spool.tile([128, NJ], F32, tag="v0")
        nc.vector.reduce_sum(
            out=v0[:],
            in_=v0parts[:].rearrange("p (cj j) -> p j cj", j=NJ),
            axis=AX.X,
        )
        nc.vector.tensor_scalar_mul(v0[:], v0[:], zcol[:, 1:2])

        # ---- gates ----
        # layer order: 0=(mul,r0) 1=(add,r0) 2=(mul,r1) 3=(add,r1)
        def gate_w1(lidx, v_bf):
            ps_h = psh.tile([128, NJ], F32, tag="psh")
            for j in range(NJ):
                for m in range(NJ):
                    nc.tensor.matmul(
                        ps_h[:, m:m + 1],
                        wg1[:, j, lidx, 128 * m:128 * (m + 1)],
                        v_bf[:, j:j + 1],
                        start=(j == 0 and m == 0),
                        stop=(j == NJ - 1 and m == NJ - 1),
                    )
            return ps_h

        def gate_rest(lidx, ps_h, out_name):
            b1c = sm[:, 0 + 8 * lidx:8 + 8 * lidx]
            gc = sm[:, 32 + 8 * lidx:40 + 8 * lidx]
            bec = sm[:, 64 + 8 * lidx:72 + 8 * lidx]

            stats = spool.tile([128, 16], F32, tag="stats", bufs=2)
            nc.vector.tensor_add(stats[:, 0:8], ps_h[:], b1c)
            nc.vector.tensor_mul(stats[:, 8:16], stats[:, 0:8], stats[:, 0:8])

            ps_st = psg.tile([1, 16], F32, tag="tiny")
            nc.tensor.matmul(ps_st[:], ones_col, stats[:])

            w4 = spool.tile([1, 16], F32, tag="w4", bufs=2)
            # mean and mean-square per level (pairs of m-columns)
            nc.vector.reduce_sum(
                out=w4[0:1, 0:4],
                in_=ps_st[0:1, 0:8].rearrange("p (l t) -> p l t", t=2),
                axis=AX.X,
            )
            nc.vector.reduce_sum(
                out=w4[0:1, 4:8],
                in_=ps_st[0:1, 8:16].rearrange("p (l t) -> p l t", t=2),
                axis=AX.X,
            )
            nc.vector.tensor_scalar_mul(w4[0:1, 0:4], w4[0:1, 0:4], 1.0 / P)
            nc.vector.tensor_scalar_mul(w4[0:1, 4:8], w4[0:1, 4:8], 1.0 / P)
            nc.vector.tensor_mul(w4[0:1, 8:12], w4[0:1, 0:4], w4[0:1, 0:4])
            nc.vector.tensor_sub(w4[0:1, 4:8], w4[0:1, 4:8], w4[0:1, 8:12])
            nc.vector.tensor_scalar_add(w4[0:1, 4:8], w4[0:1, 4:8], EPS_S)
            nc.scalar.activation(w4[0:1, 4:8], w4[0:1, 4:8], AF.Ln)
            nc.scalar.activation(
                w4[0:1, 8:12], w4[0:1, 4:8], AF.Exp, scale=-0.5
            )

            brow = spool.tile([1, 16], F32, tag="brow", bufs=2)
            bview = brow[0:1, 0:8].rearrange("p (l t) -> p t l", t=2)
            iview = brow[0:1, 8:16].rearrange("p (l t) -> p t l", t=2)
            for t in range(2):
                nc.vector.tensor_copy(bview[:, t, :], w4[0:1, 0:4])
                nc.vector.tensor_copy(iview[:, t, :], w4[0:1, 8:12])

            ps_bc = psg.tile([128, 16], F32, tag="tiny")
            nc.tensor.matmul(ps_bc[:], ones_row, brow[:])
            bc = spool.tile([128, 16], F32, tag="bc", bufs=2)
            nc.vector.tensor_copy(bc[:], ps_bc[:])

            hn = spool.tile([128, NJ], F32, tag="hn", bufs=2)
            nc.vector.tensor_sub(hn[:], stats[:, 0:8], bc[:, 0:8])
            nc.vector.tensor_mul(hn[:], hn[:], bc[:, 8:16])
            nc.vector.tensor_mul(hn[:], hn[:], gc)
            nc.vector.tensor_add(hn[:], hn[:], bec)
            hn_bf = spool.tile([128, NJ], BF, tag="hnbf", bufs=2)
            nc.scalar.activation(hn_bf[:], hn[:], AF.Relu)

            ps_z = psg.tile([128, NJ], F32, tag="tiny")
            for lv in range(4):
                for kc in range(2):
                    for clc in range(2):
                        off = lv * 512 + kc * 256 + clc * 128
                        nc.tensor.matmul(
                            ps_z[:, 2 * lv + clc:2 * lv + clc + 1],
                            wg2[:, lidx, off:off + 128],
                            hn_bf[:, 2 * lv + kc:2 * lv + kc + 1],
                            start=(lv == 0 and kc == 0 and clc == 0),
                            stop=(lv == 3 and kc == 1 and clc == 1),
                        )
            zb = spool.tile([128, NJ], F32, tag=out_name)
            b2c = sm[:, 96 + 8 * lidx:104 + 8 * lidx]
            nc.vector.tensor_add(zb[:], ps_z[:], b2c)
            return zb

        def cast_bf(src, tag):
            t = spool.tile([128, NJ], BF, tag=tag)
            nc.vector.tensor_copy(t[:], src[:])
            return t

        def sigmoid(dst, src, tag):
            tmp = spool.tile([128, NJ], F32, tag=tag, bufs=2)
            nc.scalar.activation(tmp[:], src[:], AF.Exp, scale=-1.0)
            nc.vector.tensor_scalar_add(tmp[:], tmp[:], 1.0)
            nc.vector.reciprocal(dst[:], tmp[:])

        v0_bf = cast_bf(v0, "v0bf")
        # r0: both layers' W1 back to back so the PE stays busy during LN
        h_mul0 = gate_w1(0, v0_bf)
        h_add0 = gate_w1(1, v0_bf)
        z_mul0 = gate_rest(0, h_mul0, "zmul0")
        vmul = spool.tile([128, NJ], F32, tag="vmul")
        sigmoid(vmul, z_mul0, "sigt")
        vmul_bf = cast_bf(vmul, "vmbf")
        z_add0 = gate_rest(1, h_add0, "zadd0")
        vadd = z_add0
        vadd_bf = cast_bf(vadd, "vabf")

        h_mul1 = gate_w1(2, vmul_bf)
        h_add1 = gate_w1(3, vadd_bf)
        z_mul1 = gate_rest(2, h_mul1, "zmul1")
        mm_f = spool.tile([128, NJ], F32, tag="mmf")
        sigmoid(mm_f, z_mul1, "sigt")
        nc.vector.tensor_add(mm_f[:], mm_f[:], vmul[:])
        z_add1 = gate_rest(3, h_add1, "zadd1")
        ma_f = spool.tile([128, NJ], F32, tag="maf")
        nc.vector.tensor_add(ma_f[:], z_add1[:], vadd[:])

        # ---- pass 3: output ----
        ps3 = ctx.enter_context(
            tc.tile_pool(name="ps3", bufs=4, space=bass.MemorySpace.PSUM)
        )
        addsum = spool.tile([128, 2], F32, tag="addsum")
        nc.vector.reduce_sum(
            out=addsum[:],
            in_=ma_f[:].rearrange("p (l t) -> p t l", t=2),
            axis=AX.X,
        )
        diags = []
        for js in range(NJ):
            dt_ = dpool.tile([128, 128], BF, tag=f"diag{js}", name=f"diag{js}")
            nc.vector.tensor_scalar_mul(dt_[:], rhsi[:], mm_f[:, js:js + 1])
            diags.append(dt_)

        for jj in range(2):
            for nch in range(NCH):
                ps_o = ps3.tile([128, 512], F32, tag="big")
                for lv in range(4):
                    js = 2 * lv + jj
                    nc.tensor.matmul(
                        ps_o[:],
                        diags[js][:],
                        x_sb[:, js, 512 * nch:512 * (nch + 1)],
                        start=(lv == 0), stop=(lv == 3),
                    )
                stg = stpool.tile([128, 512], BF, tag="stg")
                nc.scalar.activation(
                    stg[:], ps_o[:], AF.Identity,
                    bias=addsum[:, jj:jj + 1], scale=1.0,
                )
                nc.sync.dma_start(
                    out_d[128 * jj:128 * (jj + 1), 512 * nch:512 * (nch + 1)],
                    stg[:],
                )

    nc.compile()
    return nc


def _pack_inputs(x0, x1, x2, x3, wm, bm,
                 add_W1, add_b1, add_g, add_be, add_W2, add_b2,
                 mul_W1, mul_b1, mul_g, mul_be, mul_W2, mul_b2):
    bf = ml_dtypes.bfloat16
    f8 = ml_dtypes.float8_e4m3
    f32 = np.float32

    # shared (same for all cores)
    rhsi = np.eye(128, dtype=bf)
    ones = np.ones((128, 128), f32)
    wmv = np.asarray(wm, f32).reshape(NJ, 128)          # [j, p]
    wmr = np.broadcast_to(
        wmv.T[:, :, None], (128, NJ, 128)
    ).astype(bf).copy()

    # gate weights, layer order: (mul,0) (add,0) (mul,1) (add,1)
    W1s = [mul_W1[0], add_W1[0], mul_W1[1], add_W1[1]]
    W2s = [mul_W2[0], add_W2[0], mul_W2[1], add_W2[1]]
    b1s = [mul_b1[0], add_b1[0], mul_b1[1], add_b1[1]]
    gs = [mul_g[0], add_g[0], mul_g[1], add_g[1]]
    bes = [mul_be[0], add_be[0], mul_be[1], add_be[1]]
    b2s = [mul_b2[0], add_b2[0], mul_b2[1], add_b2[1]]

    wg1 = np.zeros((NJ, 128, 4, 1024), f8)
    wg2 = np.zeros((128, 4, 2048), bf)
    sm = np.zeros((128, 128), f32)
    for li in range(4):
        w1 = np.asarray(W1s[li], f32).reshape(C, C)       # [lp, c]
        # wg1[li, j, p, 128m+q] = w1[128m+q, 128j+p] * W1SC
        t = w1.reshape(NJ, 128, NJ, 128)                   # [m, q, j, p]
        wg1[:, :, li, :] = (
            t.transpose(2, 3, 0, 1).reshape(NJ, 128, 1024) * W1SC
        ).astype(f8)
        w2 = np.asarray(W2s[li], f32)                      # [l, cl, pp]
        # wg2[li, p, l*512+kc*256+clc*128+q] = w2[l, 128clc+q, 128kc+p]
        t2 = w2.reshape(4, 2, 128, 2, 128)                 # [l, clc, q, kc, p]
        wg2[:, li, :] = t2.transpose(4, 0, 3, 1, 2).reshape(128, 2048).astype(bf)
        sm[:, 8 * li:8 * li + 8] = (
            np.asarray(b1s[li], f32).reshape(C).reshape(NJ, 128).T * W1SC
        )
        sm[:, 32 + 8 * li:40 + 8 * li] = np.asarray(gs[li], f32).reshape(C).reshape(NJ, 128).T
        sm[:, 64 + 8 * li:72 + 8 * li] = np.asarray(bes[li], f32).reshape(C).reshape(NJ, 128).T
        b2 = np.asarray(b2s[li], f32)                      # [l, cl]
        sm[:, 96 + 8 * li:104 + 8 * li] = (
            b2.reshape(4, 2, 128).transpose(2, 0, 1).reshape(128, 8)
        )

    shared = dict(wmr=wmr, rhsi=rhsi, ones=ones, wg1=wg1, wg2=wg2, smalls=sm)

    in_maps = []
    xs = [np.asarray(a, f32) for a in (x0, x1, x2, x3)]
    for b in range(B):
        xc = np.concatenate(
            [a[b].reshape(CL, HW) for a in xs], axis=0
        ).astype(bf)
        in_maps.append({"x": xc, **shared})
    return in_maps


def kernel(**inputs):
    from concourse.bass_utils import run_bass_kernel_spmd

    if "nc" not in _CACHE:
        _CACHE["nc"] = _build_nc()
    nc = _CACHE["nc"]

    in_maps = _pack_inputs(**inputs)
    res = run_bass_kernel_spmd(nc, in_maps, list(range(NCORES)))
    _CACHE["last_results"] = res
    out = np.stack(
        [res.results[b]["out"].reshape(CL, H, W) for b in range(B)]
    ).astype(np.float32)
    return out


# revision 11
# speedup vs baseline: 1.1598x; 1.1598x over previous
# Trainium2 Bass kernel for the ContextBlock problem.
#
# Reference computation (per sample b):
#   xc    = concat(x0..x3)            [C=1024, HW=4096]
#   attn  = softmax(wm @ xc)          [HW]
#   ctx   = xc @ attn                 [C]
#   mul   = residual-gated MLP stack (sigmoid branch)   [C]
#   add   = residual-gated MLP stack (linear branch)    [C]
#   out   = sum_l (x_l * mul_l + add_l)                 [CL=256, HW]
#
# Distribution: data-parallel over batch, one sample per NeuronCore (B=8).
# No collectives required.
#
# Per-core dataflow (chunk-pipelined):
#   phase1: per 512-col chunk: logits via replicated-wm matmuls (the PSUM
#           result is partition-broadcast for free), exp on Scalar with
#           per-chunk accum (softmax denominator), then Vector
#           tensor_tensor_reduce accumulates the context dot-products.
#           Overlaps PE / Scalar / Vector / x DMA.
#   gates:  W1 in fp8 e4m3 (x64 host pre-scale; LN is scale-invariant so
#           only b1 and EPS are rescaled), W2 bf16. All weight DMA issued
#           up-front behind x. Layer pairs interleaved for PE pipelining.
#   pass3:  diag(mul) @ x accumulated over levels in PSUM, bias=add-sum,
#           bf16 output staging + DMA (host upcasts to f32).

import numpy as np
import ml_dtypes
from contextlib import ExitStack

import concourse.bass as bass
import concourse.bacc as bacc
import concourse.mybir as mybir
import concourse.tile as tile

BF = mybir.dt.bfloat16
F32 = mybir.dt.float32
F8 = mybir.dt.float8e4
AF = mybir.ActivationFunctionType
ALU = mybir.AluOpType
AX = mybir.AxisListType

B, L, CL, H, W = 8, 4, 256, 64, 64
C = L * CL          # 1024
HW = H * W          # 4096
P = C // 4          # 256
R = 2
EPS = 1e-5
W1SC = 64.0         # fp8 pre-scale for W1 (and b1); LN makes it a no-op
EPS_S = EPS * W1SC * W1SC
NJ = C // 128       # 8   c-slabs
NCH = HW // 512     # 8   512-col chunks
NCORES = 8

_CACHE = {}


def _build_nc():
    nc = bacc.Bacc()

    x_d = nc.dram_tensor("x", [C, HW], BF, kind="ExternalInput")
    wmr_d = nc.dram_tensor("wmr", [128, NJ, 128], BF, kind="ExternalInput")
    rhsi_d = nc.dram_tensor("rhsi", [128, 128], BF, kind="ExternalInput")
    ones_d = nc.dram_tensor("ones", [128, 128], F32, kind="ExternalInput")
    wg1_d = nc.dram_tensor("wg1", [NJ, 128, 4, 1024], F8, kind="ExternalInput")
    wg2_d = nc.dram_tensor("wg2", [128, 4, 2048], BF, kind="ExternalInput")
    sm_d = nc.dram_tensor("smalls", [128, 128], F32, kind="ExternalInput")
    out_d = nc.dram_tensor("out", [CL, HW], BF, kind="ExternalOutput")

    with tile.TileContext(nc) as tc, ExitStack() as ctx:
        resid = ctx.enter_context(tc.tile_pool(name="resid", bufs=1))
        spool = ctx.enter_context(tc.tile_pool(name="spool", bufs=1))
        scr = ctx.enter_context(tc.tile_pool(name="scr", bufs=2))
        stpool = ctx.enter_context(tc.tile_pool(name="stage", bufs=4))
        dpool = ctx.enter_context(tc.tile_pool(name="diag", bufs=1))
        psh = ctx.enter_context(
            tc.tile_pool(name="psh", bufs=2, space=bass.MemorySpace.PSUM)
        )
        psg = ctx.enter_context(
            tc.tile_pool(name="psg", bufs=2, space=bass.MemorySpace.PSUM)
        )

        # ---- resident tiles ----
        x_sb = resid.tile([128, NJ, HW], BF, tag="x")
        wmr = resid.tile([128, NJ, 128], BF, tag="wmr")
        rhsi = resid.tile([128, 128], BF, tag="rhsi")
        ones = resid.tile([128, 128], F32, tag="ones")
        wg1 = resid.tile([128, NJ, 4, 1024], F8, tag="wg1")
        wg2 = resid.tile([128, 4, 2048], BF, tag="wg2")
        sm = resid.tile([128, 128], F32, tag="sm")
        e_bc = resid.tile([128, NCH, 512], BF, tag="e_bc")

        # x first (half-major so phase1 can start early); weights on gpsimd
        nc.scalar.dma_start(wmr[:], wmr_d[:])
        nc.scalar.dma_start(ones[:], ones_d[:])
        nc.scalar.dma_start(sm[:], sm_d[:])
        nc.scalar.dma_start(rhsi[:], rhsi_d[:])
        for ci in range(4):
            for j in range(NJ):
                nc.sync.dma_start(
                    x_sb[:, j, 1024 * ci:1024 * (ci + 1)],
                    x_d[128 * j:128 * (j + 1), 1024 * ci:1024 * (ci + 1)],
                )
        for j in range(NJ):
            nc.sync.dma_start(wg1[:, j, :, :], wg1_d[j])
        nc.sync.dma_start(wg2[:], wg2_d[:])

        ones_col = ones[:, 0:1]
        ones_row = ones[0:1, 0:128]

        # ---- phase 1: logits (partition-broadcast) + exp + context ----
        rowsums = spool.tile([128, NCH], F32, tag="rowsums")
        v0parts = spool.tile([128, NCH * NJ], F32, tag="v0parts")
        with tc.tile_pool(name="ps1", bufs=2,
                          space=bass.MemorySpace.PSUM) as ps1:
            for cj in range(NCH):
                lgbc = ps1.tile([128, 512], F32, tag="lgbc")
                for j in range(NJ):
                    nc.tensor.matmul(
                        lgbc[:],
                        wmr[:, j, :],
                        x_sb[:, j, 512 * cj:512 * (cj + 1)],
                        start=(j == 0), stop=(j == NJ - 1),
                    )
                # e = exp(logits) broadcast over partitions; accum -> Z chunk
                nc.scalar.activation(
                    e_bc[:, cj, :], lgbc[:], AF.Exp,
                    accum_out=rowsums[:, cj:cj + 1],
                )
                for j in range(NJ):
                    eng = nc.vector
                    t = scr.tile([128, 512], BF, tag="ttrv")
                    eng.scalar_tensor_tensor(
                        out=t[:],
                        in0=x_sb[:, j, 512 * cj:512 * (cj + 1)],
                        scalar=1.0,
                        in1=e_bc[:, cj, :],
                        op0=ALU.mult,
                        op1=ALU.mult,
                        accum_out=v0parts[:, NJ * cj + j:NJ * cj + j + 1],
                    )

        zcol = spool.tile([128, 2], F32, tag="zcol")
        nc.vector.reduce_sum(out=zcol[:, 0:1], in_=rowsums[:], axis=AX.X)
        nc.vector.reciprocal(zcol[:, 1:2], zcol[:, 0:1])
        v0 = spool.tile([128, NJ], F32, tag="v0")
        nc.vector.reduce_sum(
            out=v0[:],
            in_=v0parts[:].rearrange("p (cj j) -> p j cj", j=NJ),
            axis=AX.X,
        )
        nc.vector.tensor_scalar_mul(v0[:], v0[:], zcol[:, 1:2])

        # ---- gates ----
        # layer order: 0=(mul,r0) 1=(add,r0) 2=(mul,r1) 3=(add,r1)
        def gate_w1(lidx, v_bf):
            ps_h = psh.tile([128, NJ], F32, tag="psh")
            for j in range(NJ):
                for m in range(NJ):
                    nc.tensor.matmul(
                        ps_h[:, m:m + 1],
                        wg1[:, j, lidx, 128 * m:128 * (m + 1)],
                        v_bf[:, j:j + 1],
                        start=(j == 0 and m == 0),
                        stop=(j == NJ - 1 and m == NJ - 1),
                    )
            return ps_h

        def gate_rest(lidx, ps_h, out_name):
            b1c = sm[:, 0 + 8 * lidx:8 + 8 * lidx]
            gc = sm[:, 32 + 8 * lidx:40 + 8 * lidx]
            bec = sm[:, 64 + 8 * lidx:72 + 8 * lidx]

            stats = spool.tile([128, 16], F32, tag="stats", bufs=2)
            nc.vector.tensor_add(stats[:, 0:8], ps_h[:], b1c)
            nc.vector.tensor_mul(stats[:, 8:16], stats[:, 0:8], stats[:, 0:8])

            ps_st = psg.tile([1, 16], F32, tag="tiny")
            nc.tensor.matmul(ps_st[:], ones_col, stats[:])

            w4 = spool.tile([1, 16], F32, tag="w4", bufs=2)
            # mean and mean-square per level (pairs of m-columns)
            nc.vector.reduce_sum(
                out=w4[0:1, 0:4],
                in_=ps_st[0:1, 0:8].rearrange("p (l t) -> p l t", t=2),
                axis=AX.X,
            )
            nc.vector.reduce_sum(
                out=w4[0:1, 4:8],
                in_=ps_st[0:1, 8:16].rearrange("p (l t) -> p l t", t=2),
                axis=AX.X,
            )
            nc.vector.tensor_scalar_mul(w4[0:1, 0:4], w4[0:1, 0:4], 1.0 / P)
            nc.vector.tensor_scalar_mul(w4[0:1, 4:8], w4[0:1, 4:8], 1.0 / P)
            nc.vector.tensor_mul(w4[0:1, 8:12], w4[0:1, 0:4], w4[0:1, 0:4])
            nc.vector.tensor_sub(w4[0:1, 4:8], w4[0:1, 4:8], w4[0:1, 8:12])
            nc.vector.tensor_scalar_add(w4[0:1, 4:8], w4[0:1, 4:8], EPS_S)
            nc.scalar.activation(w4[0:1, 4:8], w4[0:1, 4:8], AF.Ln)
            nc.scalar.activation(
                w4[0:1, 8:12], w4[0:1, 4:8], AF.Exp, scale=-0.5
            )

            brow = spool.tile([1, 16], F32, tag="brow", bufs=2)
            bview = brow[0:1, 0:8].rearrange("p (l t) -> p t l", t=2)
            iview = brow[0:1, 8:16].rearrange("p (l t) -> p t l", t=2)
            for t in range(2):
                nc.vector.tensor_copy(bview[:, t, :], w4[0:1, 0:4])
                nc.vector.tensor_copy(iview[:, t, :], w4[0:1, 8:12])

            ps_bc = psg.tile([128, 16], F32, tag="tiny")
            nc.tensor.matmul(ps_bc[:], ones_row, brow[:])
            bc = spool.tile([128, 16], F32, tag="bc", bufs=2)
            nc.vector.tensor_copy(bc[:], ps_bc[:])

            hn = spool.tile([128, NJ], F32, tag="hn", bufs=2)
            nc.vector.tensor_sub(hn[:], stats[:, 0:8], bc[:, 0:8])
            nc.vector.tensor_mul(hn[:], hn[:], bc[:, 8:16])
            nc.vector.tensor_mul(hn[:], hn[:], gc)
            nc.vector.tensor_add(hn[:], hn[:], bec)
            hn_bf = spool.tile([128, NJ], BF, tag="hnbf", bufs=2)
            nc.scalar.activation(hn_bf[:], hn[:], AF.Relu)

            ps_z = psg.tile([128, NJ], F32, tag="tiny")
            for lv in range(4):
                for kc in range(2):
                    for clc in range(2):
                        off = lv * 512 + kc * 256 + clc * 128
                        nc.tensor.matmul(
                            ps_z[:, 2 * lv + clc:2 * lv + clc + 1],
                            wg2[:, lidx, off:off + 128],
                            hn_bf[:, 2 * lv + kc:2 * lv + kc + 1],
                            start=(lv == 0 and kc == 0 and clc == 0),
                            stop=(lv == 3 and kc == 1 and clc == 1),
                        )
            zb = spool.tile([128, NJ], F32, tag=out_name)
            b2c = sm[:, 96 + 8 * lidx:104 + 8 * lidx]
            nc.vector.tensor_add(zb[:], ps_z[:], b2c)
            return zb

        def cast_bf(src, tag):
            t = spool.tile([128, NJ], BF, tag=tag)
            nc.vector.tensor_copy(t[:], src[:])
            return t

        def sigmoid(dst, src, tag):
            tmp = spool.tile([128, NJ], F32, tag=tag, bufs=2)
            nc.scalar.activation(tmp[:], src[:], AF.Exp, scale=-1.0)
            nc.vector.tensor_scalar_add(tmp[:], tmp[:], 1.0)
            nc.vector.reciprocal(dst[:], tmp[:])

        v0_bf = cast_bf(v0, "v0bf")
        # r0: both layers' W1 back to back so the PE stays busy during LN
        h_mul0 = gate_w1(0, v0_bf)
        h_add0 = gate_w1(1, v0_bf)
        z_mul0 = gate_rest(0, h_mul0, "zmul0")
        vmul = spool.tile([128, NJ], F32, tag="vmul")
        sigmoid(vmul, z_mul0, "sigt")
        vmul_bf = cast_bf(vmul, "vmbf")
        z_add0 = gate_rest(1, h_add0, "zadd0")
        vadd = z_add0
        vadd_bf = cast_bf(vadd, "vabf")

        h_mul1 = gate_w1(2, vmul_bf)
        h_add1 = gate_w1(3, vadd_bf)
        z_mul1 = gate_rest(2, h_mul1, "zmul1")
        mm_f = spool.tile([128, NJ], F32, tag="mmf")
        sigmoid(mm_f, z_mul1, "sigt")
        nc.vector.tensor_add(mm_f[:], mm_f[:], vmul[:])
        z_add1 = gate_rest(3, h_add1, "zadd1")
        ma_f = spool.tile([128, NJ], F32, tag="maf")
        nc.vector.tensor_add(ma_f[:], z_add1[:], vadd[:])

        # ---- pass 3: output ----
        ps3 = ctx.enter_context(
            tc.tile_pool(name="ps3", bufs=4, space=bass.MemorySpace.PSUM)
        )
        addsum = spool.tile([128, 2], F32, tag="addsum")
        nc.vector.reduce_sum(
            out=addsum[:],
            in_=ma_f[:].rearrange("p (l t) -> p t l", t=2),
            axis=AX.X,
        )
        diags = []
        for js in range(NJ):
            dt_ = dpool.tile([128, 128], BF, tag=f"diag{js}", name=f"diag{js}")
            nc.vector.tensor_scalar_mul(dt_[:], rhsi[:], mm_f[:, js:js + 1])
            diags.append(dt_)

        for jj in range(2):
            for nch in range(NCH):
                ps_o = ps3.tile([128, 512], F32, tag="big")
                for lv in range(4):
                    js = 2 * lv + jj
                    nc.tensor.matmul(
                        ps_o[:],
                        diags[js][:],
                        x_sb[:, js, 512 * nch:512 * (nch + 1)],
                        start=(lv == 0), stop=(lv == 3),
                    )
                stg = stpool.tile([128, 512], BF, tag="stg")
                nc.scalar.activation(
                    stg[:], ps_o[:], AF.Identity,
                    bias=addsum[:, jj:jj + 1], scale=1.0,
                )
                nc.sync.dma_start(
                    out_d[128 * jj:128 * (jj + 1), 512 * nch:512 * (nch + 1)],
                    stg[:],
                )

    nc.compile()
    return nc


def _pack_inputs(x0, x1, x2, x3, wm, bm,
                 add_W1, add_b1, add_g, add_be, add_W2, add_b2,
                 mul_W1, mul_b1, mul_g, mul_be, mul_W2, mul_b2):
    bf = ml_dtypes.bfloat16
    f8 = ml_dtypes.float8_e4m3
    f32 = np.float32

    # shared (same for all cores)
    rhsi = np.eye(128, dtype=bf)
    ones = np.ones((128, 128), f32)
    wmv = np.asarray(wm, f32).reshape(NJ, 128)          # [j, p]
    wmr = np.broadcast_to(
        wmv.T[:, :, None], (128, NJ, 128)
    ).astype(bf).copy()

    # gate weights, layer order: (mul,0) (add,0) (mul,1) (add,1)
    W1s = [mul_W1[0], add_W1[0], mul_W1[1], add_W1[1]]
    W2s = [mul_W2[0], add_W2[0], mul_W2[1], add_W2[1]]
    b1s = [mul_b1[0], add_b1[0], mul_b1[1], add_b1[1]]
    gs = [mul_g[0], add_g[0], mul_g[1], add_g[1]]
    bes = [mul_be[0], add_be[0], mul_be[1], add_be[1]]
    b2s = [mul_b2[0], add_b2[0], mul_b2[1], add_b2[1]]

    wg1 = np.zeros((NJ, 128, 4, 1024), f8)
    wg2 = np.zeros((128, 4, 2048), bf)
    sm = np.zeros((128, 128), f32)
    for li in range(4):
        w1 = np.asarray(W1s[li], f32).reshape(C, C)       # [lp, c]
        # wg1[li, j, p, 128m+q] = w1[128m+q, 128j+p] * W1SC
        t = w1.reshape(NJ, 128, NJ, 128)                   # [m, q, j, p]
        wg1[:, :, li, :] = (
            t.transpose(2, 3, 0, 1).reshape(NJ, 128, 1024) * W1SC
        ).astype(f8)
        w2 = np.asarray(W2s[li], f32)                      # [l, cl, pp]
        # wg2[li, p, l*512+kc*256+clc*128+q] = w2[l, 128clc+q, 128kc+p]
        t2 = w2.reshape(4, 2, 128, 2, 128)                 # [l, clc, q, kc, p]
        wg2[:, li, :] = t2.transpose(4, 0, 3, 1, 2).reshape(128, 2048).astype(bf)
        sm[:, 8 * li:8 * li + 8] = (
            np.asarray(b1s[li], f32).reshape(C).reshape(NJ, 128).T * W1SC
        )
        sm[:, 32 + 8 * li:40 + 8 * li] = np.asarray(gs[li], f32).reshape(C).reshape(NJ, 128).T
        sm[:, 64 + 8 * li:72 + 8 * li] = np.asarray(bes[li], f32).reshape(C).reshape(NJ, 128).T
        b2 = np.asarray(b2s[li], f32)                      # [l, cl]
        sm[:, 96 + 8 * li:104 + 8 * li] = (
            b2.reshape(4, 2, 128).transpose(2, 0, 1).reshape(128, 8)
        )

    shared = dict(wmr=wmr, rhsi=rhsi, ones=ones, wg1=wg1, wg2=wg2, smalls=sm)

    in_maps = []
    xs = [np.asarray(a, f32) for a in (x0, x1, x2, x3)]
    for b in range(B):
        xc = np.concatenate(
            [a[b].reshape(CL, HW) for a in xs], axis=0
        ).astype(bf)
        in_maps.append({"x": xc, **shared})
    return in_maps


def kernel(**inputs):
    from concourse.bass_utils import run_bass_kernel_spmd

    if "nc" not in _CACHE:
        _CACHE["nc"] = _build_nc()
    nc = _CACHE["nc"]

    in_maps = _pack_inputs(**inputs)
    res = run_bass_kernel_spmd(nc, in_maps, list(range(NCORES)))
    _CACHE["last_results"] = res
    out = np.stack(
        [res.results[b]["out"].reshape(CL, H, W) for b in range(B)]
    ).astype(np.float32)
    return out


# revision 12
# speedup vs baseline: 1.1963x; 1.0315x over previous
# Trainium2 Bass kernel for the ContextBlock problem.
#
# Reference computation (per sample b):
#   xc    = concat(x0..x3)            [C=1024, HW=4096]
#   attn  = softmax(wm @ xc)          [HW]
#   ctx   = xc @ attn                 [C]
#   mul   = residual-gated MLP stack (sigmoid branch)   [C]
#   add   = residual-gated MLP stack (linear branch)    [C]
#   out   = sum_l (x_l * mul_l + add_l)                 [CL=256, HW]
#
# Distribution: data-parallel over batch, one sample per NeuronCore (B=8).
# No collectives required.
#
# Per-core dataflow (chunk-pipelined):
#   phase1: per 512-col chunk: logits via replicated-wm matmuls (the PSUM
#           result is partition-broadcast for free), exp on Scalar with
#           per-chunk accum (softmax denominator), then Vector
#           tensor_tensor_reduce accumulates the context dot-products.
#           Overlaps PE / Scalar / Vector / x DMA.
#   gates:  W1 in fp8 e4m3 (x64 host pre-scale; LN is scale-invariant so
#           only b1 and EPS are rescaled), W2 bf16. All weight DMA issued
#           up-front behind x. Layer pairs interleaved for PE pipelining.
#   pass3:  diag(mul) @ x accumulated over levels in PSUM, bias=add-sum,
#           bf16 output staging + DMA (host upcasts to f32).

import numpy as np
import ml_dtypes
from contextlib import ExitStack

import concourse.bass as bass
import concourse.bacc as bacc
import concourse.mybir as mybir
import concourse.tile as tile

BF = mybir.dt.bfloat16
F32 = mybir.dt.float32
F8 = mybir.dt.float8e4
AF = mybir.ActivationFunctionType
ALU = mybir.AluOpType
AX = mybir.AxisListType

B, L, CL, H, W = 8, 4, 256, 64, 64
C = L * CL          # 1024
HW = H * W          # 4096
P = C // 4          # 256
R = 2
EPS = 1e-5
W1SC = 64.0         # fp8 pre-scale for W1 (and b1); LN makes it a no-op
EPS_S = EPS * W1SC * W1SC
NJ = C // 128       # 8   c-slabs
NCH = HW // 512     # 8   512-col chunks
NCORES = 8

_CACHE = {}


class _Bacc(bacc.Bacc):
    """Bacc whose act-table pass prefers the exp+ln set.

    The stock pass greedily assigns each activation the first table set
    containing its function (Exp -> set 0, Ln -> set 5), inserting a
    1.28us ACT_TABLE_LOAD at every Exp<->Ln transition on the serial
    gate chain. Every function this kernel uses (Exp, Ln, Relu,
    Identity) lives in 'natural_log_exp_and_others', so present that set
    first, then remap the emitted ids back to canonical act_info order.
    """

    def insert_act_table_loads(self):
        import bass_rust as _bass_rust
        from concourse.hw_specs import get_activation_tables

        has_activation = any(
            isinstance(i, mybir.InstActivation)
            for b in self.main_func.blocks
            for i in b.instructions
        )
        if not has_activation:
            return
        items = list(get_activation_tables(self.m.arch).items())
        pref = next(
            i for i, (n, _) in enumerate(items)
            if n == "natural_log_exp_and_others"
        )
        reordered = [items[pref]] + items[:pref] + items[pref + 1:]
        remap = {0: pref}
        for pos, (name, _) in enumerate(reordered[1:], start=1):
            remap[pos] = items.index((name, dict(items)[name]))
        _bass_rust.insert_act_table_loads(self, reordered)
        for b in self.main_func.blocks:
            for i in b.instructions:
                if isinstance(i, mybir.InstLoadActFuncSet):
                    i.act_func_set_id = remap[i.act_func_set_id]


def _build_nc():
    nc = _Bacc()

    x_d = nc.dram_tensor("x", [C, HW], BF, kind="ExternalInput")
    wmr_d = nc.dram_tensor("wmr", [128, NJ, 128], BF, kind="ExternalInput")
    rhsi_d = nc.dram_tensor("rhsi", [128, 128], BF, kind="ExternalInput")
    ones_d = nc.dram_tensor("ones", [128, 128], F32, kind="ExternalInput")
    wg1_d = nc.dram_tensor("wg1", [NJ, 128, 4, 1024], F8, kind="ExternalInput")
    wg2_d = nc.dram_tensor("wg2", [128, 4, 2048], BF, kind="ExternalInput")
    sm_d = nc.dram_tensor("smalls", [128, 128], F32, kind="ExternalInput")
    out_d = nc.dram_tensor("out", [CL, HW], BF, kind="ExternalOutput")

    with tile.TileContext(nc) as tc, ExitStack() as ctx:
        resid = ctx.enter_context(tc.tile_pool(name="resid", bufs=1))
        spool = ctx.enter_context(tc.tile_pool(name="spool", bufs=1))
        scr = ctx.enter_context(tc.tile_pool(name="scr", bufs=2))
        stpool = ctx.enter_context(tc.tile_pool(name="stage", bufs=4))
        dpool = ctx.enter_context(tc.tile_pool(name="diag", bufs=1))
        psh = ctx.enter_context(
            tc.tile_pool(name="psh", bufs=2, space=bass.MemorySpace.PSUM)
        )
        psg = ctx.enter_context(
            tc.tile_pool(name="psg", bufs=2, space=bass.MemorySpace.PSUM)
        )

        # ---- resident tiles ----
        x_sb = resid.tile([128, NJ, HW], BF, tag="x")
        wmr = resid.tile([128, NJ, 128], BF, tag="wmr")
        rhsi = resid.tile([128, 128], BF, tag="rhsi")
        ones = resid.tile([128, 128], F32, tag="ones")
        wg1 = resid.tile([128, NJ, 4, 1024], F8, tag="wg1")
        wg2 = resid.tile([128, 4, 2048], BF, tag="wg2")
        sm = resid.tile([128, 128], F32, tag="sm")
        e_bc = resid.tile([128, NCH, 512], BF, tag="e_bc")

        # x first (half-major so phase1 can start early); weights on gpsimd
        nc.scalar.dma_start(wmr[:], wmr_d[:])
        nc.scalar.dma_start(ones[:], ones_d[:])
        nc.scalar.dma_start(sm[:], sm_d[:])
        nc.scalar.dma_start(rhsi[:], rhsi_d[:])
        for ci in range(4):
            for j in range(NJ):
                nc.sync.dma_start(
                    x_sb[:, j, 1024 * ci:1024 * (ci + 1)],
                    x_d[128 * j:128 * (j + 1), 1024 * ci:1024 * (ci + 1)],
                )
        for j in range(NJ):
            nc.sync.dma_start(wg1[:, j, :, :], wg1_d[j])
        nc.sync.dma_start(wg2[:], wg2_d[:])

        ones_col = ones[:, 0:1]
        ones_row = ones[0:1, 0:128]

        # ---- phase 1: logits (partition-broadcast) + exp + context ----
        rowsums = spool.tile([128, NCH], F32, tag="rowsums")
        v0parts = spool.tile([128, NCH * NJ], F32, tag="v0parts")
        with tc.tile_pool(name="ps1", bufs=2,
                          space=bass.MemorySpace.PSUM) as ps1:
            for cj in range(NCH):
                lgbc = ps1.tile([128, 512], F32, tag="lgbc")
                for j in range(NJ):
                    nc.tensor.matmul(
                        lgbc[:],
                        wmr[:, j, :],
                        x_sb[:, j, 512 * cj:512 * (cj + 1)],
                        start=(j == 0), stop=(j == NJ - 1),
                    )
                # e = exp(logits) broadcast over partitions; accum -> Z chunk
                nc.scalar.activation(
                    e_bc[:, cj, :], lgbc[:], AF.Exp,
                    accum_out=rowsums[:, cj:cj + 1],
                )
                for j in range(NJ):
                    eng = nc.vector
                    t = scr.tile([128, 512], BF, tag="ttrv")
                    eng.scalar_tensor_tensor(
                        out=t[:],
                        in0=x_sb[:, j, 512 * cj:512 * (cj + 1)],
                        scalar=1.0,
                        in1=e_bc[:, cj, :],
                        op0=ALU.mult,
                        op1=ALU.mult,
                        accum_out=v0parts[:, NJ * cj + j:NJ * cj + j + 1],
                    )

        zcol = spool.tile([128, 2], F32, tag="zcol")
        nc.vector.reduce_sum(out=zcol[:, 0:1], in_=rowsums[:], axis=AX.X)
        nc.vector.reciprocal(zcol[:, 1:2], zcol[:, 0:1])
        v0 = spool.tile([128, NJ], F32, tag="v0")
        nc.vector.reduce_sum(
            out=v0[:],
            in_=v0parts[:].rearrange("p (cj j) -> p j cj", j=NJ),
            axis=AX.X,
        )
        nc.vector.tensor_scalar_mul(v0[:], v0[:], zcol[:, 1:2])

        # ---- gates ----
        # layer order: 0=(mul,r0) 1=(add,r0) 2=(mul,r1) 3=(add,r1)
        def gate_w1(lidx, v_bf):
            ps_h = psh.tile([128, NJ], F32, tag="psh")
            for j in range(NJ):
                for m in range(NJ):
                    nc.tensor.matmul(
                        ps_h[:, m:m + 1],
                        wg1[:, j, lidx, 128 * m:128 * (m + 1)],
                        v_bf[:, j:j + 1],
                        start=(j == 0 and m == 0),
                        stop=(j == NJ - 1 and m == NJ - 1),
                    )
            return ps_h

        def gate_rest(lidx, ps_h, out_name):
            b1c = sm[:, 0 + 8 * lidx:8 + 8 * lidx]
            gc = sm[:, 32 + 8 * lidx:40 + 8 * lidx]
            bec = sm[:, 64 + 8 * lidx:72 + 8 * lidx]

            stats = spool.tile([128, 16], F32, tag="stats", bufs=2)
            nc.vector.tensor_add(stats[:, 0:8], ps_h[:], b1c)
            nc.vector.tensor_mul(stats[:, 8:16], stats[:, 0:8], stats[:, 0:8])

            ps_st = psg.tile([1, 16], F32, tag="tiny")
            nc.tensor.matmul(ps_st[:], ones_col, stats[:])

            w4 = spool.tile([1, 16], F32, tag="w4", bufs=2)
            # mean and mean-square per level (pairs of m-columns)
            nc.vector.reduce_sum(
                out=w4[0:1, 0:4],
                in_=ps_st[0:1, 0:8].rearrange("p (l t) -> p l t", t=2),
                axis=AX.X,
            )
            nc.vector.reduce_sum(
                out=w4[0:1, 4:8],
                in_=ps_st[0:1, 8:16].rearrange("p (l t) -> p l t", t=2),
                axis=AX.X,
            )
            nc.vector.tensor_scalar_mul(w4[0:1, 0:4], w4[0:1, 0:4], 1.0 / P)
            nc.vector.tensor_scalar_mul(w4[0:1, 4:8], w4[0:1, 4:8], 1.0 / P)
            nc.vector.tensor_mul(w4[0:1, 8:12], w4[0:1, 0:4], w4[0:1, 0:4])
            nc.vector.tensor_sub(w4[0:1, 4:8], w4[0:1, 4:8], w4[0:1, 8:12])
            nc.vector.tensor_scalar_add(w4[0:1, 4:8], w4[0:1, 4:8], EPS_S)
            nc.scalar.activation(w4[0:1, 4:8], w4[0:1, 4:8], AF.Ln)
            nc.scalar.activation(
                w4[0:1, 8:12], w4[0:1, 4:8], AF.Exp, scale=-0.5
            )

            brow = spool.tile([1, 16], F32, tag="brow", bufs=2)
            bview = brow[0:1, 0:8].rearrange("p (l t) -> p t l", t=2)
            iview = brow[0:1, 8:16].rearrange("p (l t) -> p t l", t=2)
            for t in range(2):
                nc.vector.tensor_copy(bview[:, t, :], w4[0:1, 0:4])
                nc.vector.tensor_copy(iview[:, t, :], w4[0:1, 8:12])

            ps_bc = psg.tile([128, 16], F32, tag="tiny")
            nc.tensor.matmul(ps_bc[:], ones_row, brow[:])
            bc = spool.tile([128, 16], F32, tag="bc", bufs=2)
            nc.vector.tensor_copy(bc[:], ps_bc[:])

            hn = spool.tile([128, NJ], F32, tag="hn", bufs=2)
            nc.vector.tensor_sub(hn[:], stats[:, 0:8], bc[:, 0:8])
            nc.vector.tensor_mul(hn[:], hn[:], bc[:, 8:16])
            nc.vector.tensor_mul(hn[:], hn[:], gc)
            nc.vector.tensor_add(hn[:], hn[:], bec)
            hn_bf = spool.tile([128, NJ], BF, tag="hnbf", bufs=2)
            nc.scalar.activation(hn_bf[:], hn[:], AF.Relu)

            ps_z = psg.tile([128, NJ], F32, tag="tiny")
            for lv in range(4):
                for kc in range(2):
                    for clc in range(2):
                        off = lv * 512 + kc * 256 + clc * 128
                        nc.tensor.matmul(
                            ps_z[:, 2 * lv + clc:2 * lv + clc + 1],
                            wg2[:, lidx, off:off + 128],
                            hn_bf[:, 2 * lv + kc:2 * lv + kc + 1],
                            start=(lv == 0 and kc == 0 and clc == 0),
                            stop=(lv == 3 and kc == 1 and clc == 1),
                        )
            zb = spool.tile([128, NJ], F32, tag=out_name)
            b2c = sm[:, 96 + 8 * lidx:104 + 8 * lidx]
            nc.vector.tensor_add(zb[:], ps_z[:], b2c)
            return zb

        def cast_bf(src, tag):
            t = spool.tile([128, NJ], BF, tag=tag)
            nc.vector.tensor_copy(t[:], src[:])
            return t

        def sigmoid(dst, src, tag):
            tmp = spool.tile([128, NJ], F32, tag=tag, bufs=2)
            nc.scalar.activation(tmp[:], src[:], AF.Exp, scale=-1.0)
            nc.vector.tensor_scalar_add(tmp[:], tmp[:], 1.0)
            nc.vector.reciprocal(dst[:], tmp[:])

        v0_bf = cast_bf(v0, "v0bf")
        # r0: both layers' W1 back to back so the PE stays busy during LN
        h_mul0 = gate_w1(0, v0_bf)
        h_add0 = gate_w1(1, v0_bf)
        z_mul0 = gate_rest(0, h_mul0, "zmul0")
        vmul = spool.tile([128, NJ], F32, tag="vmul")
        sigmoid(vmul, z_mul0, "sigt")
        vmul_bf = cast_bf(vmul, "vmbf")
        z_add0 = gate_rest(1, h_add0, "zadd0")
        vadd = z_add0
        vadd_bf = cast_bf(vadd, "vabf")

        h_mul1 = gate_w1(2, vmul_bf)
        h_add1 = gate_w1(3, vadd_bf)
        z_mul1 = gate_rest(2, h_mul1, "zmul1")
        mm_f = spool.tile([128, NJ], F32, tag="mmf")
        sigmoid(mm_f, z_mul1, "sigt")
        nc.vector.tensor_add(mm_f[:], mm_f[:], vmul[:])
        z_add1 = gate_rest(3, h_add1, "zadd1")
        ma_f = spool.tile([128, NJ], F32, tag="maf")
        nc.vector.tensor_add(ma_f[:], z_add1[:], vadd[:])

        # ---- pass 3: output ----
        ps3 = ctx.enter_context(
            tc.tile_pool(name="ps3", bufs=4, space=bass.MemorySpace.PSUM)
        )
        addsum = spool.tile([128, 2], F32, tag="addsum")
        nc.vector.reduce_sum(
            out=addsum[:],
            in_=ma_f[:].rearrange("p (l t) -> p t l", t=2),
            axis=AX.X,
        )
        diags = []
        for js in range(NJ):
            dt_ = dpool.tile([128, 128], BF, tag=f"diag{js}", name=f"diag{js}")
            nc.vector.tensor_scalar_mul(dt_[:], rhsi[:], mm_f[:, js:js + 1])
            diags.append(dt_)

        for jj in range(2):
            for nch in range(NCH):
                ps_o = ps3.tile([128, 512], F32, tag="big")
                for lv in range(4):
                    js = 2 * lv + jj
                    nc.tensor.matmul(
                        ps_o[:],
                        diags[js][:],
                        x_sb[:, js, 512 * nch:512 * (nch + 1)],
                        start=(lv == 0), stop=(lv == 3),
                    )
                stg = stpool.tile([128, 512], BF, tag="stg")
                nc.scalar.activation(
                    stg[:], ps_o[:], AF.Identity,
                    bias=addsum[:, jj:jj + 1], scale=1.0,
                )
                nc.sync.dma_start(
                    out_d[128 * jj:128 * (jj + 1), 512 * nch:512 * (nch + 1)],
                    stg[:],
                )

    nc.compile()
    return nc


def _pack_inputs(x0, x1, x2, x3, wm, bm,
                 add_W1, add_b1, add_g, add_be, add_W2, add_b2,
                 mul_W1, mul_b1, mul_g, mul_be, mul_W2, mul_b2):
    bf = ml_dtypes.bfloat16
    f8 = ml_dtypes.float8_e4m3
    f32 = np.float32

    # shared (same for all cores)
    rhsi = np.eye(128, dtype=bf)
    ones = np.ones((128, 128), f32)
    wmv = np.asarray(wm, f32).reshape(NJ, 128)          # [j, p]
    wmr = np.broadcast_to(
        wmv.T[:, :, None], (128, NJ, 128)
    ).astype(bf).copy()

    # gate weights, layer order: (mul,0) (add,0) (mul,1) (add,1)
    W1s = [mul_W1[0], add_W1[0], mul_W1[1], add_W1[1]]
    W2s = [mul_W2[0], add_W2[0], mul_W2[1], add_W2[1]]
    b1s = [mul_b1[0], add_b1[0], mul_b1[1], add_b1[1]]
    gs = [mul_g[0], add_g[0], mul_g[1], add_g[1]]
    bes = [mul_be[0], add_be[0], mul_be[1], add_be[1]]
    b2s = [mul_b2[0], add_b2[0], mul_b2[1], add_b2[1]]

    wg1 = np.zeros((NJ, 128, 4, 1024), f8)
    wg2 = np.zeros((128, 4, 2048), bf)
    sm = np.zeros((128, 128), f32)
    for li in range(4):
        w1 = np.asarray(W1s[li], f32).reshape(C, C)       # [lp, c]
        # wg1[li, j, p, 128m+q] = w1[128m+q, 128j+p] * W1SC
        t = w1.reshape(NJ, 128, NJ, 128)                   # [m, q, j, p]
        wg1[:, :, li, :] = (
            t.transpose(2, 3, 0, 1).reshape(NJ, 128, 1024) * W1SC
        ).astype(f8)
        w2 = np.asarray(W2s[li], f32)                      # [l, cl, pp]
        # wg2[li, p, l*512+kc*256+clc*128+q] = w2[l, 128clc+q, 128kc+p]
        t2 = w2.reshape(4, 2, 128, 2, 128)                 # [l, clc, q, kc, p]
        wg2[:, li, :] = t2.transpose(4, 0, 3, 1, 2).reshape(128, 2048).astype(bf)
        sm[:, 8 * li:8 * li + 8] = (
            np.asarray(b1s[li], f32).reshape(C).reshape(NJ, 128).T * W1SC
        )
        sm[:, 32 + 8 * li:40 + 8 * li] = np.asarray(gs[li], f32).reshape(C).reshape(NJ, 128).T
        sm[:, 64 + 8 * li:72 + 8 * li] = np.asarray(bes[li], f32).reshape(C).reshape(NJ, 128).T
        b2 = np.asarray(b2s[li], f32)                      # [l, cl]
        sm[:, 96 + 8 * li:104 + 8 * li] = (
            b2.reshape(4, 2, 128).transpose(2, 0, 1).reshape(128, 8)
        )

    shared = dict(wmr=wmr, rhsi=rhsi, ones=ones, wg1=wg1, wg2=wg2, smalls=sm)

    in_maps = []
    xs = [np.asarray(a, f32) for a in (x0, x1, x2, x3)]
    for b in range(B):
        xc = np.concatenate(
            [a[b].reshape(CL, HW) for a in xs], axis=0
        ).astype(bf)
        in_maps.append({"x": xc, **shared})
    return in_maps


def kernel(**inputs):
    from concourse.bass_utils import run_bass_kernel_spmd

    if "nc" not in _CACHE:
        _CACHE["nc"] = _build_nc()
    nc = _CACHE["nc"]

    in_maps = _pack_inputs(**inputs)
    res = run_bass_kernel_spmd(nc, in_maps, list(range(NCORES)))
    _CACHE["last_results"] = res
    out = np.stack(
        [res.results[b]["out"].reshape(CL, H, W) for b in range(B)]
    ).astype(np.float32)
    return out
